# revision 1
# baseline (speedup 1.0000x reference)
"""CapsLayer kernel v3: j-sharded, 4-way column-tiled fp32 contraction.

Math: the reference's routing loop is dead (softmax over a size-1 axis is
identically 1), so the output is
    s[b, j, l] = sum_{i,k} W[i, j, l, k] * inputs[b, i, k]
    vj = squash(s, axis=l)  ->  [B, 1, NUM_CAPS, DIM_CAPS]

Sharding: W splits over NUM_CAPS j (4 capsules / 128 output columns per
core, 16.8 MB of W each); inputs (4 MB) are replicated.  Everything stays
on-core — no collectives (an 8-core ncfw ReduceScatter measures ~42 us of
fixed latency, far more than the 4 MB of duplicated input traffic costs).

PE: the contraction runs as 16 accumulation chains (one per k), assigned
round-robin to the four 32-column PE array groups via tile_position, so
four fp32 matmul streams are in flight concurrently and the per-
instruction overhead + fp32 double-pass cost is hidden.  Chain g
accumulates into PSUM partitions [32g, 32g+32).  A final 128x32 identity-
block matmul (E packed into tile 0's rows) folds the four partial chains
into s[b, n], and squash runs on [B=32, 128].

Raw Bass: this walrus build rejects instructions carrying 2+ sem waits, so
all sync is standalone wait_ge ops.  DVE/ACT same-engine RAW needs explicit
semaphores (the pipelines do not interlock through SBUF).
"""

from contextlib import ExitStack

import numpy as np

B = 32
IN_CAPS = 2048
IN_DIM = 16
NUM_CAPS = 32
DIM_CAPS = 32
NCORES = 8
JPC = NUM_CAPS // NCORES          # 4 capsules per core
NJL = JPC * DIM_CAPS              # 128 output columns per core
P = 128
NTILES = IN_CAPS // P             # 16
XROW = IN_DIM * B                 # 512 packed x floats per row (k, b)
WROW = NJL * IN_DIM               # 2048 packed w floats per row (j, l, k)
EROW = B                          # 32 identity-block floats per row
ROW = XROW + WROW + EROW          # 2592
NG = 4                            # PE column groups
EPS = 1e-7

_CACHE = {}


def _build():
    import concourse.bass as bass
    from concourse import mybir

    f32 = mybir.dt.float32
    nc = bass.Bass()
    xw = nc.declare_dram_parameter("xw", [IN_CAPS, ROW], f32, isOutput=False)
    out = nc.declare_dram_parameter("out", [B, NJL], f32, isOutput=True)

    with ExitStack() as ctx:
        xw_sb = ctx.enter_context(nc.sbuf_tensor([P, NTILES * ROW], f32))
        p4_sb = ctx.enter_context(nc.sbuf_tensor([P, NJL], f32))
        sv = ctx.enter_context(nc.sbuf_tensor([B, NJL], f32))
        sq = ctx.enter_context(nc.sbuf_tensor([B, NJL], f32))
        ss = ctx.enter_context(nc.sbuf_tensor([B, JPC], f32))
        rt = ctx.enter_context(nc.sbuf_tensor([B, JPC], f32))
        a1 = ctx.enter_context(nc.sbuf_tensor([B, JPC], f32))
        den = ctx.enter_context(nc.sbuf_tensor([B, JPC], f32))
        rden = ctx.enter_context(nc.sbuf_tensor([B, JPC], f32))
        fsc = ctx.enter_context(nc.sbuf_tensor([B, JPC], f32))
        epst = ctx.enter_context(nc.sbuf_tensor([B, 1], f32))
        warm = ctx.enter_context(nc.sbuf_tensor([B, 1], f32))
        vout = ctx.enter_context(nc.sbuf_tensor([B, NJL], f32))
        ps4 = ctx.enter_context(nc.psum_tensor([P, NJL], f32))
        pss = ctx.enter_context(nc.psum_tensor([B, NJL], f32))

        tsem = [ctx.enter_context(nc.semaphore(f"t{t}")) for t in range(NTILES)]
        pe_sem = ctx.enter_context(nc.semaphore("pe"))
        act_sem = ctx.enter_context(nc.semaphore("act"))
        dve_sem = ctx.enter_context(nc.semaphore("dve"))
        odma = ctx.enter_context(nc.semaphore("odma"))
        block = ctx.enter_context(nc.Block())

        @block.sync
        def _(sync):
            for t in range(NTILES):
                sync.dma_start(
                    out=xw_sb[:, t * ROW:(t + 1) * ROW],
                    in_=xw[t * P:(t + 1) * P, :],
                ).then_inc(tsem[t], 16)
            sync.wait_ge(dve_sem, 7)
            sync.dma_start(out=out[:, :], in_=vout[:, :]).then_inc(odma, 16)
            sync.wait_ge(odma, 16)

        @block.tensor
        def _(tensor):
            for t in range(NTILES):
                base = t * ROW
                tensor.wait_ge(tsem[t], 16)
                wview = xw_sb[:, base + XROW:base + XROW + WROW].rearrange(
                    "p (n k) -> p n k", k=IN_DIM
                )
                for k in range(IN_DIM):
                    g = k % NG
                    mm = nc.tensor.matmul(
                        ps4[32 * g:32 * (g + 1), :],
                        xw_sb[:, base + k * B:base + (k + 1) * B],
                        wview[:, :, k],
                        start=(t == 0 and k < NG),
                        stop=(t == NTILES - 1 and k >= IN_DIM - NG),
                        tile_position=(0, 32 * g),
                    )
            mm.then_inc(pe_sem, 1)
            # merge the 4 partial chains: s[b, n] = sum_g p4[32g+b, n]
            tensor.wait_ge(dve_sem, 1)
            nc.tensor.matmul(
                pss[:, :],
                xw_sb[:, XROW + WROW:ROW],       # E block from tile 0
                p4_sb[:, :],
                start=True,
                stop=True,
            ).then_inc(pe_sem, 1)

        @block.vector
        def _(vector):
            nc.vector.memset(epst[:, :], EPS)
            vector.wait_ge(pe_sem, 1)
            nc.vector.tensor_copy(p4_sb[:, :], ps4[:, :]).then_inc(dve_sem, 1)
            # squash: sq = sv^2, ss[g] = sum over each DIM_CAPS group
            vector.wait_ge(act_sem, 1)
            nc.vector.tensor_mul(sq[:, :], sv[:, :], sv[:, :]).then_inc(dve_sem, 1)
            vector.wait_ge(dve_sem, 2)
            red = nc.vector.reduce_sum(
                out=ss[:, :],
                in_=sq[:, :].rearrange("p (g d) -> p g d", g=JPC),
                axis=mybir.AxisListType.X,
            )
            red.then_inc(dve_sem, 1)
            vector.wait_ge(act_sem, 2)
            nc.vector.tensor_mul(den[:, :], a1[:, :], rt[:, :]).then_inc(dve_sem, 1)
            vector.wait_ge(dve_sem, 4)
            nc.vector.reciprocal(out=rden[:, :], in_=den[:, :]).then_inc(dve_sem, 1)
            vector.wait_ge(dve_sem, 5)
            nc.vector.tensor_mul(fsc[:, :], ss[:, :], rden[:, :]).then_inc(
                dve_sem, 1
            )
            vector.wait_ge(dve_sem, 6)
            for g in range(JPC):
                tsm = nc.vector.tensor_scalar_mul(
                    out=vout[:, g * DIM_CAPS:(g + 1) * DIM_CAPS],
                    in0=sv[:, g * DIM_CAPS:(g + 1) * DIM_CAPS],
                    scalar1=fsc[:, g:g + 1],
                )
            tsm.then_inc(dve_sem, 1)

        @block.scalar
        def _(scalar):
            # dummy Sqrt at t=0 pulls the ~1.3us ACT table load off the
            # epilogue critical path (operands are a scratch tile nobody
            # else touches; the value is unused)
            nc.scalar.activation(
                out=warm[:, :], in_=warm[:, :],
                func=mybir.ActivationFunctionType.Sqrt, bias=warm[:, :],
            )
            scalar.wait_ge(pe_sem, 2)
            nc.scalar.copy(out=sv[:, :], in_=pss[:, :]).then_inc(act_sem, 1)
            scalar.wait_ge(dve_sem, 3)
            nc.scalar.activation(
                out=rt[:, :], in_=ss[:, :],
                func=mybir.ActivationFunctionType.Sqrt, bias=epst[:, :],
            )
            nc.scalar.activation(
                out=a1[:, :], in_=ss[:, :],
                func=mybir.ActivationFunctionType.Copy, bias=1.0,
            ).then_inc(act_sem, 1)

    return nc


def _in_maps(inputs, W):
    x_t = np.transpose(inputs, (1, 2, 0)).reshape(IN_CAPS, XROW)  # [i, (k, b)]
    erow = np.zeros((IN_CAPS, B), dtype=np.float32)
    erow[np.arange(IN_CAPS), np.arange(IN_CAPS) % B] = 1.0       # E[p%32 == b]
    maps = []
    for c in range(NCORES):
        xwc = np.empty((IN_CAPS, ROW), dtype=np.float32)
        xwc[:, :XROW] = x_t
        xwc[:, XROW:XROW + WROW] = W[:, c * JPC:(c + 1) * JPC].reshape(
            IN_CAPS, WROW
        )
        xwc[:, XROW + WROW:] = erow
        maps.append({"xw": xwc})
    return maps


def kernel(inputs, W):
    from concourse.bass_utils import run_bass_kernel_spmd

    inputs = np.asarray(inputs, dtype=np.float32)
    W = np.asarray(W, dtype=np.float32)
    if "nc" not in _CACHE:
        _CACHE["nc"] = _build()
    res = run_bass_kernel_spmd(_CACHE["nc"], _in_maps(inputs, W), list(range(NCORES)))
    return np.concatenate(
        [res.results[c]["out"].reshape(B, 1, JPC, DIM_CAPS) for c in range(NCORES)],
        axis=2,
    )



# revision 4
# speedup vs baseline: 1.5909x; 1.5909x over previous
"""CapsLayer kernel v4: j-sharded, 4-way column-tiled bf16 contraction.

Math: the reference's routing loop is dead (softmax over a size-1 axis is
identically 1), so the output is
    s[b, j, l] = sum_{i,k} W[i, j, l, k] * inputs[b, i, k]
    vj = squash(s, axis=l)  ->  [B, 1, NUM_CAPS, DIM_CAPS]

Sharding: W splits over NUM_CAPS j (4 capsules / 128 output columns per
core); inputs are replicated.  Everything stays on-core — no collectives
(an 8-core ncfw ReduceScatter measures ~42 us of fixed latency, far more
than the duplicated input traffic costs).

Precision: x/W/E stream in bf16 (10.6 MB/core instead of 21.2 fp32 — the
v3 trace showed the DMA stream already at the ~360 B/ns bus limit, so
halving bytes halves the stream), PSUM accumulation and the squash run in
fp32.  End-to-end rel err ~2.6e-3 vs the 2e-2 gate.  Matmuls drop the
fp32 LOW/HIGH double pass, so the PE stream stays ahead of the DMA.

PE: the contraction runs as 16 accumulation chains (one per k), assigned
round-robin to the four 32-column PE array groups via tile_position, so
four fp32 matmul streams are in flight concurrently and the per-
instruction overhead + fp32 double-pass cost is hidden.  Chain g
accumulates into PSUM partitions [32g, 32g+32).  A final 128x32 identity-
block matmul (E packed into tile 0's rows) folds the four partial chains
into s[b, n], and squash runs on [B=32, 128].

Raw Bass: this walrus build rejects instructions carrying 2+ sem waits, so
all sync is standalone wait_ge ops.  DVE/ACT same-engine RAW needs explicit
semaphores (the pipelines do not interlock through SBUF).
"""

from contextlib import ExitStack

import numpy as np

B = 32
IN_CAPS = 2048
IN_DIM = 16
NUM_CAPS = 32
DIM_CAPS = 32
NCORES = 8
JPC = NUM_CAPS // NCORES          # 4 capsules per core
NJL = JPC * DIM_CAPS              # 128 output columns per core
P = 128
NTILES = IN_CAPS // P             # 16
XROW = IN_DIM * B                 # 512 packed x floats per row (k, b)
WROW = NJL * IN_DIM               # 2048 packed w floats per row (j, l, k)
EROW = B                          # 32 identity-block floats per row
ROW = XROW + WROW + EROW          # 2592
NG = 4                            # PE column groups
EPS = 1e-7

_CACHE = {}


def _build():
    import concourse.bass as bass
    from concourse import mybir

    f32 = mybir.dt.float32
    bf16 = mybir.dt.bfloat16
    nc = bass.Bass()
    xw = nc.declare_dram_parameter("xw", [IN_CAPS, ROW], bf16, isOutput=False)
    out = nc.declare_dram_parameter("out", [B, NJL], f32, isOutput=True)

    with ExitStack() as ctx:
        xw_sb = ctx.enter_context(nc.sbuf_tensor([P, NTILES * ROW], bf16))
        p4_sb = ctx.enter_context(nc.sbuf_tensor([P, NJL], bf16))
        sv = ctx.enter_context(nc.sbuf_tensor([B, NJL], f32))
        sq = ctx.enter_context(nc.sbuf_tensor([B, NJL], f32))
        ss = ctx.enter_context(nc.sbuf_tensor([B, JPC], f32))
        rt = ctx.enter_context(nc.sbuf_tensor([B, JPC], f32))
        a1 = ctx.enter_context(nc.sbuf_tensor([B, JPC], f32))
        den = ctx.enter_context(nc.sbuf_tensor([B, JPC], f32))
        rden = ctx.enter_context(nc.sbuf_tensor([B, JPC], f32))
        fsc = ctx.enter_context(nc.sbuf_tensor([B, JPC], f32))
        epst = ctx.enter_context(nc.sbuf_tensor([B, 1], f32))
        warm = ctx.enter_context(nc.sbuf_tensor([B, 1], f32))
        vout = ctx.enter_context(nc.sbuf_tensor([B, NJL], f32))
        ps4 = ctx.enter_context(nc.psum_tensor([P, NJL], f32))
        pss = ctx.enter_context(nc.psum_tensor([B, NJL], f32))

        tsem = [ctx.enter_context(nc.semaphore(f"t{t}")) for t in range(NTILES)]
        pe_sem = ctx.enter_context(nc.semaphore("pe"))
        act_sem = ctx.enter_context(nc.semaphore("act"))
        dve_sem = ctx.enter_context(nc.semaphore("dve"))
        odma = ctx.enter_context(nc.semaphore("odma"))
        block = ctx.enter_context(nc.Block())

        @block.sync
        def _(sync):
            for t in range(NTILES):
                sync.dma_start(
                    out=xw_sb[:, t * ROW:(t + 1) * ROW],
                    in_=xw[t * P:(t + 1) * P, :],
                ).then_inc(tsem[t], 16)
            sync.wait_ge(dve_sem, 7)
            sync.dma_start(out=out[:, :], in_=vout[:, :]).then_inc(odma, 16)
            sync.wait_ge(odma, 16)

        @block.tensor
        def _(tensor):
            for t in range(NTILES):
                base = t * ROW
                tensor.wait_ge(tsem[t], 16)
                wview = xw_sb[:, base + XROW:base + XROW + WROW].rearrange(
                    "p (n k) -> p n k", k=IN_DIM
                )
                for k in range(IN_DIM):
                    g = k % NG
                    mm = nc.tensor.matmul(
                        ps4[32 * g:32 * (g + 1), :],
                        xw_sb[:, base + k * B:base + (k + 1) * B],
                        wview[:, :, k],
                        start=(t == 0 and k < NG),
                        stop=(t == NTILES - 1 and k >= IN_DIM - NG),
                        tile_position=(0, 32 * g),
                    )
            mm.then_inc(pe_sem, 1)
            # merge the 4 partial chains: s[b, n] = sum_g p4[32g+b, n]
            tensor.wait_ge(dve_sem, 1)
            nc.tensor.matmul(
                pss[:, :],
                xw_sb[:, XROW + WROW:ROW],       # E block from tile 0
                p4_sb[:, :],
                start=True,
                stop=True,
            ).then_inc(pe_sem, 1)

        @block.vector
        def _(vector):
            nc.vector.memset(epst[:, :], EPS)
            vector.wait_ge(pe_sem, 1)
            nc.vector.tensor_copy(p4_sb[:, :], ps4[:, :]).then_inc(dve_sem, 1)
            # squash: sq = sv^2, ss[g] = sum over each DIM_CAPS group
            vector.wait_ge(act_sem, 1)
            nc.vector.tensor_mul(sq[:, :], sv[:, :], sv[:, :]).then_inc(dve_sem, 1)
            vector.wait_ge(dve_sem, 2)
            red = nc.vector.reduce_sum(
                out=ss[:, :],
                in_=sq[:, :].rearrange("p (g d) -> p g d", g=JPC),
                axis=mybir.AxisListType.X,
            )
            red.then_inc(dve_sem, 1)
            vector.wait_ge(act_sem, 2)
            nc.vector.tensor_mul(den[:, :], a1[:, :], rt[:, :]).then_inc(dve_sem, 1)
            vector.wait_ge(dve_sem, 4)
            nc.vector.reciprocal(out=rden[:, :], in_=den[:, :]).then_inc(dve_sem, 1)
            vector.wait_ge(dve_sem, 5)
            nc.vector.tensor_mul(fsc[:, :], ss[:, :], rden[:, :]).then_inc(
                dve_sem, 1
            )
            vector.wait_ge(dve_sem, 6)
            for g in range(JPC):
                tsm = nc.vector.tensor_scalar_mul(
                    out=vout[:, g * DIM_CAPS:(g + 1) * DIM_CAPS],
                    in0=sv[:, g * DIM_CAPS:(g + 1) * DIM_CAPS],
                    scalar1=fsc[:, g:g + 1],
                )
            tsm.then_inc(dve_sem, 1)

        @block.scalar
        def _(scalar):
            # dummy Sqrt at t=0 pulls the ~1.3us ACT table load off the
            # epilogue critical path (operands are a scratch tile nobody
            # else touches; the value is unused)
            nc.scalar.activation(
                out=warm[:, :], in_=warm[:, :],
                func=mybir.ActivationFunctionType.Sqrt, bias=warm[:, :],
            )
            scalar.wait_ge(pe_sem, 2)
            nc.scalar.copy(out=sv[:, :], in_=pss[:, :]).then_inc(act_sem, 1)
            scalar.wait_ge(dve_sem, 3)
            nc.scalar.activation(
                out=rt[:, :], in_=ss[:, :],
                func=mybir.ActivationFunctionType.Sqrt, bias=epst[:, :],
            )
            nc.scalar.activation(
                out=a1[:, :], in_=ss[:, :],
                func=mybir.ActivationFunctionType.Copy, bias=1.0,
            ).then_inc(act_sem, 1)

    return nc


def _in_maps(inputs, W):
    import ml_dtypes

    bf = np.dtype(ml_dtypes.bfloat16)
    x_t = np.ascontiguousarray(
        np.transpose(inputs, (1, 2, 0)).reshape(IN_CAPS, XROW)
    ).astype(bf)                                                 # [i, (k, b)]
    W16 = W.astype(bf)
    erow = np.zeros((IN_CAPS, B), dtype=bf)
    erow[np.arange(IN_CAPS), np.arange(IN_CAPS) % B] = 1.0       # E[p%32 == b]
    maps = []
    for c in range(NCORES):
        xwc = np.empty((IN_CAPS, ROW), dtype=bf)
        xwc[:, :XROW] = x_t
        xwc[:, XROW:XROW + WROW] = W16[:, c * JPC:(c + 1) * JPC].reshape(
            IN_CAPS, WROW
        )
        xwc[:, XROW + WROW:] = erow
        maps.append({"xw": xwc})
    return maps


def kernel(inputs, W):
    from concourse.bass_utils import run_bass_kernel_spmd

    inputs = np.asarray(inputs, dtype=np.float32)
    W = np.asarray(W, dtype=np.float32)
    if "nc" not in _CACHE:
        _CACHE["nc"] = _build()
    res = run_bass_kernel_spmd(_CACHE["nc"], _in_maps(inputs, W), list(range(NCORES)))
    return np.concatenate(
        [res.results[c]["out"].reshape(B, 1, JPC, DIM_CAPS) for c in range(NCORES)],
        axis=2,
    )

